# revision 1
# baseline (speedup 1.0000x reference)
"""Multi-head attention (L=2048, EMB=1024, H=16, D=64) on 8 TRN2 NeuronCores.

Tensor-parallel over heads: core i owns heads {2i, 2i+1} (a 128-row block of
Wq/Wk/Wv and a 128-column block of Wo). Each core computes its two heads'
attention plus its partial output projection; the host sums the 8 partials.

Device-side layout is fully transposed (scores^T = [m, l]) so no on-device
transposes are needed:
  QT[d, l] = (Wq_shard @ q^T)        lhsT = (Wq_shard/8)^T, rhs = q^T
  KT[d, l] = (Wk_shard @ k^T)
  V [m, d] = (v @ Wv_shard^T)        lhsT = v^T tile,       rhs = Wv_shard^T
  sT[m, l] = KT_h^T @ QT_h           (per head, contraction d=64)
  pT       = exp(sT) * keepT         (no max-subtraction: |s| <~ 9)
  attnT/Z  = [V_h | 1]^T @ pT        (ones column gives softmax denominator)
  outT     = Wo_shard^T-block @ (attnT / Z)   partial, summed on host

All matmuls run in bf16 (fp32 PSUM accumulation); measured end-to-end
relative error vs the fp32 reference is ~0.6%.

Pipeline structure (all tuned against neuron-profile NTFF traces):
- l-tile-major attention; each l-tile's epilogue (softmax-denominator
  normalize + output projection + store) is deferred and drip-fed as
  filler into the NEXT l-tile's quad stream, so the multi-hop Z DMA
  chain never blocks the in-order engine queues.
- exp() is batched over 3 key-tiles per ACTIVATE ((N+352)/1.2ns cost).
- One-stage software pipeline on the PE queue: quad q's attn matmuls
  are emitted after quad q+1's scores, decoupling PE from the
  exp->mask-mult chain.
- K/V/Q projection rounds are interleaved into the first head's
  attention stream; input DMAs are split across the sync/scalar/gpsimd
  rings in consumption order; mask chunks prefetch one head ahead.
- The softmax reciprocal is spread across 8 partitions via a DRAM
  bounce (single-partition DVE reciprocal costs ~6ns/element).
"""

import sys

for _p in ("/opt/trn_rl_repo",):
    if _p not in sys.path:
        sys.path.insert(0, _p)

from contextlib import ExitStack

import ml_dtypes
import numpy as np

import concourse.bass as bass
import concourse.tile as tile
from concourse import bacc, mybir
from concourse._compat import with_exitstack
from concourse.bass_utils import run_bass_kernel_spmd

BF16 = mybir.dt.bfloat16
FP8 = mybir.dt.float8e4
F32 = mybir.dt.float32
NPBF16 = ml_dtypes.bfloat16

L = 2048
EMB = 1024
NHEAD = 16
HEAD_DIM = 64
NCORES = 8
HPC = NHEAD // NCORES  # heads per core = 2
ROWS = HPC * HEAD_DIM  # weight rows per core = 128
SCALE = HEAD_DIM ** -0.5

LT = 512               # l-tile (matmul free dim / PSUM bank)
NLT = L // LT          # 4
MT = 128               # m-tile (key-block on partitions)
NMT = L // MT          # 16
ET = 128               # contraction tile over EMB
NET = EMB // ET        # 8
JT = 128               # output-row tile
NJT = EMB // JT        # 8

QUADS = (3, 3, 3, 3, 2, 2)   # m-tiles per exp/mask-mult instruction
QB = 3                        # psc tile m-capacity (PSUM banks per slot)
PSC_BUFS = 2


@with_exitstack
def _mha_kernel(ctx, tc, outT, qT, kT, vT, wqT, wkT, wvT, woT, maskT, dbg=None):
    nc = tc.nc

    const = ctx.enter_context(tc.tile_pool(name="const", bufs=1))
    ppool = ctx.enter_context(tc.tile_pool(name="ptiles", bufs=5))
    maskp = ctx.enter_context(tc.tile_pool(name="maskp", bufs=2))
    stage = ctx.enter_context(tc.tile_pool(name="stage", bufs=4))
    zpool = ctx.enter_context(tc.tile_pool(name="zpool", bufs=2))
    psc = ctx.enter_context(tc.tile_pool(name="psc", bufs=PSC_BUFS, space="PSUM"))
    psa = ctx.enter_context(tc.tile_pool(name="psa", bufs=2, space="PSUM"))

    # ---- resident input tiles; DMAs emitted in consumption order ----
    qTs = const.tile([128, NET, L], BF16, tag="qTs")
    kTs = const.tile([128, NET, L], BF16, tag="kTs")
    vTs = const.tile([128, NET, L], BF16, tag="vTs")
    wqs = const.tile([128, NET, ROWS], BF16, tag="wqs")
    wks = const.tile([128, NET, ROWS], BF16, tag="wks")
    wvs = const.tile([128, NET, ROWS], BF16, tag="wvs")
    wos = const.tile([128, EMB], BF16, tag="wos")  # [hd, j]
    q3 = qT.rearrange("(o p) l -> p o l", p=128)
    k3 = kT.rearrange("(o p) l -> p o l", p=128)
    v3 = vT.rearrange("(o p) l -> p o l", p=128)

    def chunk_dma(dst, src3, lc):
        nc.sync.dma_start(dst[:, :, bass.ts(lc, LT)], src3[:, :, bass.ts(lc, LT)])

    state = {}
    mask3 = maskT.rearrange("h (mo p) l -> h p mo l", p=128)

    def mask_fetch(lt, h, eng=None):
        mc = maskp.tile([128, NMT, LT], BF16, tag="maskc", name=f"maskc_{lt}_{h}")
        (eng or nc.scalar).dma_start(mc[:], mask3[h, :, :, bass.ts(lt, LT)])
        state[lt, h, "maskc"] = mc

    def chunk_dma_eng(eng, dst, src3, lc):
        eng.dma_start(dst[:, :, bass.ts(lc, LT)], src3[:, :, bass.ts(lc, LT)])

    # critical first chunks on the scalar HWDGE ring (short queue)
    nc.scalar.dma_start(wqs[:], wqT[:])
    chunk_dma_eng(nc.scalar, qTs, q3, 0)
    nc.scalar.dma_start(wks[:], wkT[:])
    chunk_dma_eng(nc.scalar, kTs, k3, 0)
    nc.scalar.dma_start(wvs[:], wvT[:])
    chunk_dma_eng(nc.scalar, vTs, v3, 0)
    # bulk tail chunks on the gpsimd SWDGE ring
    for lc in (2, 3):
        chunk_dma_eng(nc.gpsimd, kTs, k3, lc)
        chunk_dma_eng(nc.gpsimd, vTs, v3, lc)
    mask_fetch(0, 0, eng=nc.sync)
    # the rest on sync, behind the first mask chunk
    chunk_dma_eng(nc.sync, kTs, k3, 1)
    chunk_dma_eng(nc.sync, vTs, v3, 1)
    for lc in range(1, NLT):
        chunk_dma_eng(nc.sync, qTs, q3, lc)
    nc.sync.dma_start(wos[:], woT[:])

    QTb = const.tile([128, L], BF16, tag="QTb")
    KTb = const.tile([128, L], BF16, tag="KTb")
    VROW = 66
    vaug = const.tile([128, HPC, NMT, VROW], BF16, tag="vaug")
    nc.vector.memset(vaug[:, :, :, HEAD_DIM : HEAD_DIM + 1], 1.0)
    nc.vector.memset(vaug[:, :, :, HEAD_DIM + 1 : VROW], 0.0)

    def qk_proj(dst, w, x, lt):
        ps = psc.tile([128, QB, LT], F32, tag="psc", name="ps_proj")[:, 0, :]
        for et in range(NET):
            nc.tensor.matmul(
                ps[:],
                lhsT=w[:, et, :],
                rhs=x[:, et, bass.ts(lt, LT)],
                start=(et == 0),
                stop=(et == NET - 1),
            )
        nc.vector.tensor_copy(out=dst[:, bass.ts(lt, LT)], in_=ps[:])

    def v_proj(mt):
        ps = psc.tile([128, QB, LT], F32, tag="psc", name="ps_v")[:, 0, :ROWS]
        for et in range(NET):
            nc.tensor.matmul(
                ps[:],
                lhsT=vTs[:, et, bass.ts(mt, MT)],
                rhs=wvs[:, et, :],
                start=(et == 0),
                stop=(et == NET - 1),
            )
        for h in range(HPC):
            nc.vector.tensor_copy(
                out=vaug[:, h, mt, 0:HEAD_DIM],
                in_=ps[:, bass.ts(h, HEAD_DIM)],
            )

    # ---- attention + per-l-tile epilogue ----
    attnTb = const.tile([128, L], BF16, tag="attnTb")

    zdram = nc.dram_tensor("zdram", [NLT, HPC * LT], F32).ap()
    zidram = nc.dram_tensor("zidram", [NLT, HPC * LT], F32).ap()

    # Epilogue work for l-tile X is deferred and drip-fed as PE/DVE filler
    # into l-tile X+1's quad stream, so the z-chain DMA latency never sits
    # in front of the in-order engine queues.
    pending = []

    def piece_zload(lt):
        def go():
            zsp = zpool.tile([8, HPC * LT // 8], F32, tag="zsp", name=f"zsp_{lt}")
            nc.sync.dma_start(zsp[:], zdram[lt].rearrange("(o p) -> o p", o=8))
            state[lt, "zsp"] = zsp
        return go

    def piece_recip_bcast(lt):
        def go():
            zsp = state[lt, "zsp"]
            nc.vector.reciprocal(zsp[:], zsp[:])
            nc.sync.dma_start(zidram[lt].rearrange("(o p) -> o p", o=8), zsp[:])
            zinvb = zpool.tile([128, LT], F32, tag="zinvb", name=f"zinvb_{lt}")
            for h in range(HPC):
                nc.sync.dma_start(
                    zinvb[bass.ts(h, HEAD_DIM), :],
                    zidram[lt][None, bass.ts(h, LT)].to_broadcast((HEAD_DIM, LT)),
                )
            state[lt, "zinvb"] = zinvb
        return go

    def piece_norm(lt):
        def go():
            ls = bass.ts(lt, LT)
            nc.vector.tensor_mul(
                out=attnTb[:, ls], in0=attnTb[:, ls], in1=state[lt, "zinvb"][:]
            )
        return go

    def piece_outproj(lt, jt):
        def go():
            ls = bass.ts(lt, LT)
            ps = psc.tile([128, QB, LT], F32, tag="psc", name="ps_out")[:, 0, :]
            nc.tensor.matmul(
                ps[:],
                lhsT=wos[:, bass.ts(jt, JT)],
                rhs=attnTb[:, ls],
                start=True,
                stop=True,
            )
            st = stage.tile([128, LT], F32, tag="st", name="st")
            nc.vector.tensor_copy(out=st[:], in_=ps[:])
            nc.gpsimd.dma_start(outT[bass.ts(jt, JT), ls], st[:])
        return go

    qk_proj(QTb, wqs, qTs, 0)

    def mask_fetch(lt, h, eng=None):
        mc = maskp.tile([128, NMT, LT], BF16, tag="maskc", name=f"maskc_{lt}_{h}")
        (eng or nc.scalar).dma_start(mc[:], mask3[h, :, :, bass.ts(lt, LT)])
        state[lt, h, "maskc"] = mc


    for lt in range(NLT):
        ls = bass.ts(lt, LT)
        zseg = zpool.tile([128, HPC * LT], F32, tag="zseg", name=f"zseg_{lt}")
        for h in range(HPC):
            hd = bass.ts(h, HEAD_DIM)
            nxt = lt * HPC + h + 1
            if nxt < NLT * HPC:
                mask_fetch(nxt // HPC, nxt % HPC)
            maskc = state[lt, h, "maskc"]
            pa = psa.tile([128, LT], F32, tag="psa", name=f"psa_{lt}_{h}")
            mt0 = 0
            prev_attn = None
            for qi, qn in enumerate(QUADS):
                if lt == 0 and h == 0:
                    # interleave K/V projection rounds into the first
                    # attention stream so the PE queue never drains
                    for mt in range(mt0, mt0 + qn):
                        if mt % (LT // MT) == 0:
                            qk_proj(KTb, wks, kTs, mt // (LT // MT))
                        v_proj(mt)
                if lt == 0 and h == 1 and 1 <= qi <= 3:
                    qk_proj(QTb, wqs, qTs, qi)  # PE filler + needed later
                if pending:
                    pending[0][0] -= 1
                    if pending[0][0] < 0:
                        pending.pop(0)[1]()
                ss = psc.tile([128, QB, LT], F32, tag="psc", name="ss")
                for i in range(qn):
                    nc.tensor.matmul(
                        ss[:, i, :],
                        lhsT=KTb[hd, bass.ts(mt0 + i, MT)],
                        rhs=QTb[hd, ls],
                        start=True,
                        stop=True,
                    )
                # one-stage software pipeline on PE: the previous quad's
                # attn matmuls are emitted AFTER this quad's scores, so the
                # in-order PE queue never blocks scores behind the
                # exp->mask-mult chain of the previous quad
                if prev_attn is not None:
                    prev_attn()
                pT = ppool.tile([128, QB, LT], BF16, tag="pT", name="pT")
                nc.scalar.activation(
                    pT[:, :qn, :], ss[:, :qn, :], mybir.ActivationFunctionType.Exp
                )
                nc.vector.tensor_mul(
                    out=pT[:, :qn, :], in0=pT[:, :qn, :],
                    in1=maskc[:, mt0 : mt0 + qn, :],
                )

                def make_attn(mt0=mt0, qn=qn, pT=pT):
                    def go():
                        for i in range(qn):
                            mt = mt0 + i
                            nc.tensor.matmul(
                                pa[:VROW, :],
                                lhsT=vaug[:, h, mt, :],
                                rhs=pT[:, i, :],
                                start=(mt == 0),
                                stop=(mt == NMT - 1),
                            )
                    return go

                prev_attn = make_attn()
                mt0 += qn
            prev_attn()
            nc.vector.tensor_copy(out=attnTb[hd, ls], in_=pa[0:HEAD_DIM, :])
            nc.vector.tensor_copy(
                out=zseg[HEAD_DIM : HEAD_DIM + 1, bass.ts(h, LT)],
                in_=pa[HEAD_DIM : HEAD_DIM + 1, :],
            )
        nc.sync.dma_start(zdram[lt][None, :], zseg[HEAD_DIM : HEAD_DIM + 1, :])
        pending.append([1, piece_zload(lt)])
        pending.append([0, piece_recip_bcast(lt)])
        pending.append([2, piece_norm(lt)])
        for jt in range(NJT):
            pending.append([0, piece_outproj(lt, jt)])

    while pending:
        pending.pop(0)[1]()

    if dbg is not None:
        nc.sync.dma_start(dbg["QTb"][:], QTb[:])
        nc.sync.dma_start(dbg["KTb"][:], KTb[:])
        nc.sync.dma_start(dbg["vaug"][:], vaug[:])
        nc.sync.dma_start(dbg["attnTb_post"][:], attnTb[:])


_CACHE = {}


def _build(debug=False):
    key = ("nc", debug)
    if key in _CACHE:
        return _CACHE[key]
    nc = bacc.Bacc("TRN2", target_bir_lowering=False, debug=False,
                   num_devices=NCORES)
    qT = nc.dram_tensor("qT", [EMB, L], BF16, kind="ExternalInput").ap()
    kT = nc.dram_tensor("kT", [EMB, L], BF16, kind="ExternalInput").ap()
    vT = nc.dram_tensor("vT", [EMB, L], BF16, kind="ExternalInput").ap()
    wqT = nc.dram_tensor("wqT", [128, NET, ROWS], BF16, kind="ExternalInput").ap()
    wkT = nc.dram_tensor("wkT", [128, NET, ROWS], BF16, kind="ExternalInput").ap()
    wvT = nc.dram_tensor("wvT", [128, NET, ROWS], BF16, kind="ExternalInput").ap()
    woT = nc.dram_tensor("woT", [ROWS, EMB], BF16, kind="ExternalInput").ap()
    maskT = nc.dram_tensor("maskT", [HPC, L, L], BF16, kind="ExternalInput").ap()
    outT = nc.dram_tensor("outT", [EMB, L], F32, kind="ExternalOutput").ap()
    dbg = None
    if debug:
        dbg = {
            "QTb": nc.dram_tensor("dbg_QTb", [128, L], BF16, kind="ExternalOutput").ap(),
            "KTb": nc.dram_tensor("dbg_KTb", [128, L], BF16, kind="ExternalOutput").ap(),
            "vaug": nc.dram_tensor("dbg_vaug", [128, NMT, HPC, HEAD_DIM + 1], BF16, kind="ExternalOutput").ap(),
            "attnTb_pre": nc.dram_tensor("dbg_attnTb_pre", [128, L], BF16, kind="ExternalOutput").ap(),
            "attnTb_post": nc.dram_tensor("dbg_attnTb_post", [128, L], BF16, kind="ExternalOutput").ap(),
            "zinvb": nc.dram_tensor("dbg_zinvb", [128, L], F32, kind="ExternalOutput").ap(),
        }

    with tile.TileContext(nc) as tc:
        _mha_kernel(tc, outT, qT, kT, vT, wqT, wkT, wvT, woT, maskT, dbg=dbg)
    nc.compile()
    _CACHE[key] = nc
    return nc


def _pack_w(w):
    # [ROWS, EMB] -> w.T [EMB, ROWS] -> [128, NET, ROWS] with e = o*128+p
    return np.ascontiguousarray(
        w.T.reshape(NET, 128, ROWS).transpose(1, 0, 2)
    ).astype(NPBF16)


def _prep_in_maps(q, k, v, mask, Wq, Wk, Wv, Wo):
    qT = np.ascontiguousarray(q.T).astype(NPBF16)
    kT = np.ascontiguousarray(k.T).astype(NPBF16)
    vT = np.ascontiguousarray(v.T).astype(NPBF16)
    in_maps = []
    for c in range(NCORES):
        rows = slice(c * ROWS, (c + 1) * ROWS)
        in_maps.append({
            "qT": qT,
            "kT": kT,
            "vT": vT,
            "wqT": _pack_w(Wq[rows] * SCALE),
            "wkT": _pack_w(Wk[rows]),
            "wvT": _pack_w(Wv[rows]),
            "woT": np.ascontiguousarray(Wo[:, rows].T).astype(NPBF16),
            "maskT": np.ascontiguousarray(
                (~mask[c * HPC : (c + 1) * HPC]).swapaxes(1, 2)
            ).astype(NPBF16),
        })
    return in_maps


def run(q, k, v, mask, Wq, Wk, Wv, Wo, debug=False, **spmd_kwargs):
    nc = _build(debug=debug)
    in_maps = _prep_in_maps(q, k, v, mask, Wq, Wk, Wv, Wo)
    res = run_bass_kernel_spmd(nc, in_maps, list(range(NCORES)), **spmd_kwargs)
    outT = np.zeros((EMB, L), np.float64)
    for r in res.results:
        outT += r["outT"].astype(np.float64)
    out = np.ascontiguousarray(outT.T).astype(np.float32)
    return out, res


def kernel(q, k, v, mask, Wq, Wk, Wv, Wo):
    q, k, v = (np.asarray(x, np.float32) for x in (q, k, v))
    Wq, Wk, Wv, Wo = (np.asarray(x, np.float32) for x in (Wq, Wk, Wv, Wo))
    mask = np.asarray(mask, bool)
    out, _ = run(q, k, v, mask, Wq, Wk, Wv, Wo)
    return out



# revision 4
# speedup vs baseline: 1.1591x; 1.1591x over previous
"""Multi-head attention (L=2048, EMB=1024, H=16, D=64) on 8 TRN2 NeuronCores.

Tensor-parallel over heads: core i owns heads {2i, 2i+1} (a 128-row block of
Wq/Wk/Wv and a 128-column block of Wo). Each core computes its two heads'
attention plus its partial output projection; the host sums the 8 partials.

Device-side layout is fully transposed (scores^T = [m, l]) so no on-device
transposes are needed:
  QT[d, l] = (Wq_shard @ q^T)        lhsT = (Wq_shard/8)^T, rhs = q^T
  KT[d, l] = (Wk_shard @ k^T)
  V [m, d] = (v @ Wv_shard^T)        lhsT = v^T tile,       rhs = Wv_shard^T
  sT[m, l] = KT_h^T @ QT_h           (per head, contraction d=64)
  sT      += -30 * mask  (fp8 mask applied ON THE PE: one extra matmul per
                          score tile with lhsT = -30*I, rhs = fp8 mask tile,
                          accumulating into the score PSUM bank)
  pT       = exp(sT)                 (masked entries become e^-30*e^s ~ 0)
  attnT/Z  = [V_h | 1]^T @ pT        (ones column gives softmax denominator)
  outT     = Wo_shard^T-block @ (attnT / Z)   partial (bf16), summed on host

Matmuls in bf16 (fp32 PSUM accumulation); mask matmuls in fp8e4.

Performance structure (driven by the PE p-state ramp: the tensor engine only
reaches max clock after ~3us of gap-free execution, so the whole kernel is
organized to keep the PE queue continuously fed):
- ~14 warmup matmuls on a zeroed tile ramp the PE from t=0 while the first
  input DMAs land.
- Scores run 2 quads ahead of the attention matmuls (lag-2 software
  pipeline) so the PE never waits on the exp chain.
- Mask application is on the PE itself (see above): the only cross-engine
  chain is PE -> scalar(exp) -> PE.
- K/V/Q projection rounds interleave into the first two head-blocks; each
  l-tile's epilogue (z-reciprocal chain + output projection) drips into the
  NEXT l-tile's quad stream as PE filler.
- Input DMAs are consumption-ordered across the sync/scalar HWDGE queues and
  the gpsimd SWDGE ring; the fp8 mask chunks prefetch one head-block ahead.
- The softmax reciprocal is spread across 8 partitions via a DRAM bounce
  (single-partition DVE reciprocal costs ~6ns/element).
"""

import sys

for _p in ("/opt/trn_rl_repo",):
    if _p not in sys.path:
        sys.path.insert(0, _p)

from contextlib import ExitStack

import ml_dtypes
import numpy as np

import concourse.bass as bass
import concourse.tile as tile
from concourse import bacc, mybir
from concourse._compat import with_exitstack
from concourse.bass_utils import run_bass_kernel_spmd

BF16 = mybir.dt.bfloat16
FP8 = mybir.dt.float8e4
F32 = mybir.dt.float32
NPBF16 = ml_dtypes.bfloat16
NPFP8 = ml_dtypes.float8_e4m3fn

L = 2048
EMB = 1024
NHEAD = 16
HEAD_DIM = 64
NCORES = 8
HPC = NHEAD // NCORES  # heads per core = 2
ROWS = HPC * HEAD_DIM  # weight rows per core = 128
SCALE = HEAD_DIM ** -0.5

LT = 512               # l-tile (matmul free dim / PSUM bank)
NLT = L // LT          # 4
MT = 128               # m-tile (key-block on partitions)
NMT = L // MT          # 16
ET = 128               # contraction tile over EMB
NET = EMB // ET        # 8
JT = 128               # output-row tile
NJT = EMB // JT        # 8

QUADS = (3, 3, 3, 3, 2, 2)   # m-tiles per exp instruction
QB = 3                        # psc tile m-capacity (PSUM banks per slot)
ATTN_LAG = 2                  # quads the attn matmuls trail the scores
NWARM = 14                    # PE warmup matmuls (p-state ramp)
MASK_NEG = -30.0              # additive mask magnitude (exp(-30+9) ~ 1e-10)


@with_exitstack
def _mha_kernel(ctx, tc, outT, qT, kT, vT, wqT, wkT, wvT, woT, maskT, negIT):
    nc = tc.nc

    const = ctx.enter_context(tc.tile_pool(name="const", bufs=1))
    ppool = ctx.enter_context(tc.tile_pool(name="ptiles", bufs=5))
    maskp = ctx.enter_context(tc.tile_pool(name="maskp", bufs=2))
    stage = ctx.enter_context(tc.tile_pool(name="stage", bufs=3))
    zpool = ctx.enter_context(tc.tile_pool(name="zpool", bufs=2))
    psc = ctx.enter_context(tc.tile_pool(name="psc", bufs=2, space="PSUM"))
    psa = ctx.enter_context(tc.tile_pool(name="psa", bufs=2, space="PSUM"))

    # ---- resident tiles ----
    qTs = const.tile([128, NET, L], BF16, tag="qTs")
    kTs = const.tile([128, NET, L], BF16, tag="kTs")
    vTs = const.tile([128, NET, L], BF16, tag="vTs")
    wqs = const.tile([128, NET, ROWS], BF16, tag="wqs")
    wks = const.tile([128, NET, ROWS], BF16, tag="wks")
    wvs = const.tile([128, NET, ROWS], BF16, tag="wvs")
    wos = const.tile([128, EMB], BF16, tag="wos")  # [hd, j]
    negI = const.tile([128, 128], FP8, tag="negI")
    wz = const.tile([128, 640], BF16, tag="wz")  # warmup zeros
    q3 = qT.rearrange("(o p) l -> p o l", p=128)
    k3 = kT.rearrange("(o p) l -> p o l", p=128)
    v3 = vT.rearrange("(o p) l -> p o l", p=128)
    mask3 = maskT.rearrange("h (mo p) l -> h p mo l", p=128)

    state = {}

    def mask_fetch(lt, h):
        mc = maskp.tile([128, NMT, LT], FP8, tag="maskc", name=f"maskc_{lt}_{h}")
        nc.gpsimd.dma_start(mc[:], mask3[h, :, :, bass.ts(lt, LT)])
        state[lt, h, "maskc"] = mc

    def chunk_dma(eng, dst, src3, lc):
        eng.dma_start(dst[:, :, bass.ts(lc, LT)], src3[:, :, bass.ts(lc, LT)])

    # warmup input: zeroed by gpsimd at t=0 (no dependencies)
    nc.gpsimd.memset(wz[:], 0.0)
    # ring (gpsimd SWDGE, async): mask chunks + non-critical weights
    nc.gpsimd.dma_start(negI[:], negIT[:])
    mask_fetch(0, 0)
    nc.gpsimd.dma_start(wvs[:], wvT[:])
    mask_fetch(0, 1)
    nc.gpsimd.dma_start(wos[:], woT[:])
    # scalar HWDGE: K-projection inputs + v0 (idle until the first exp)
    nc.scalar.dma_start(wks[:], wkT[:])
    chunk_dma(nc.scalar, kTs, k3, 0)
    chunk_dma(nc.scalar, vTs, v3, 0)
    # sync HWDGE: Q-projection critical path, then remaining chunks in
    # consumption order
    nc.sync.dma_start(wqs[:], wqT[:])
    chunk_dma(nc.sync, qTs, q3, 0)
    for lc in (1, 2, 3):
        chunk_dma(nc.sync, kTs, k3, lc)
    for lc in (1, 2, 3):
        chunk_dma(nc.sync, vTs, v3, lc)
    for lc in (1, 2, 3):
        chunk_dma(nc.sync, qTs, q3, lc)

    QTb = const.tile([128, L], BF16, tag="QTb")
    KTb = const.tile([128, L], BF16, tag="KTb")
    VROW = 66
    vaug = const.tile([128, HPC, NMT, VROW], BF16, tag="vaug")
    nc.vector.memset(vaug[:, :, :, HEAD_DIM : HEAD_DIM + 1], 1.0)
    nc.vector.memset(vaug[:, :, :, HEAD_DIM + 1 : VROW], 0.0)

    # ---- PE warmup: ramp the p-state from t=0 on zero data ----
    psw = psc.tile([128, QB, LT], F32, tag="psc", name="ps_warm")
    for i in range(NWARM):
        nc.tensor.matmul(
            psw[:, i % 2, :], lhsT=wz[:, :128], rhs=wz[:, 128:640],
            start=True, stop=True,
        )

    def qk_proj(dst, w, x, lt):
        ps = psc.tile([128, QB, LT], F32, tag="psc", name="ps_proj")[:, 0, :]
        for et in range(NET):
            nc.tensor.matmul(
                ps[:],
                lhsT=w[:, et, :],
                rhs=x[:, et, bass.ts(lt, LT)],
                start=(et == 0),
                stop=(et == NET - 1),
            )
        nc.vector.tensor_copy(out=dst[:, bass.ts(lt, LT)], in_=ps[:])

    def v_proj_tri(mt0, n):
        # n (<=3) m-tiles of the V projection into one psc buf; one copy out
        ps = psc.tile([128, QB, LT], F32, tag="psc", name="ps_v")
        for i in range(n):
            for et in range(NET):
                nc.tensor.matmul(
                    ps[:, i, :ROWS],
                    lhsT=vTs[:, et, bass.ts(mt0 + i, MT)],
                    rhs=wvs[:, et, :],
                    start=(et == 0),
                    stop=(et == NET - 1),
                )
        # ps[:, i, h*64:(h+1)*64] -> vaug[:, h, mt0+i, 0:64]
        src = ps[:, 0:n, :ROWS].rearrange("p n (h d) -> p n h d", h=HPC)
        dst = vaug[:, :, mt0 : mt0 + n, 0:HEAD_DIM].rearrange(
            "p h n d -> p n h d"
        )
        nc.vector.tensor_copy(out=dst, in_=src)

    # ---- attention + per-l-tile epilogue ----
    attnTb = const.tile([128, L], BF16, tag="attnTb")

    zdram = nc.dram_tensor("zdram", [NLT, HPC * LT], F32).ap()
    zidram = nc.dram_tensor("zidram", [NLT, HPC * LT], F32).ap()

    # Epilogue work for l-tile X is deferred and drip-fed as PE/DVE filler
    # into l-tile X+1's quad stream, so the z-chain DMA latency never sits
    # in front of the in-order engine queues.
    pending = []

    def piece_zload(lt):
        def go():
            zsp = zpool.tile([8, HPC * LT // 8], F32, tag="zsp", name=f"zsp_{lt}")
            nc.sync.dma_start(zsp[:], zdram[lt].rearrange("(o p) -> o p", o=8))
            state[lt, "zsp"] = zsp
        return go

    def piece_recip_bcast(lt):
        def go():
            zsp = state[lt, "zsp"]
            nc.vector.reciprocal(zsp[:], zsp[:])
            nc.sync.dma_start(zidram[lt].rearrange("(o p) -> o p", o=8), zsp[:])
            zinvb = zpool.tile([128, LT], F32, tag="zinvb", name=f"zinvb_{lt}")
            for h in range(HPC):
                nc.sync.dma_start(
                    zinvb[bass.ts(h, HEAD_DIM), :],
                    zidram[lt][None, bass.ts(h, LT)].to_broadcast((HEAD_DIM, LT)),
                )
            state[lt, "zinvb"] = zinvb
        return go

    def piece_norm(lt):
        def go():
            ls = bass.ts(lt, LT)
            nc.vector.tensor_mul(
                out=attnTb[:, ls], in0=attnTb[:, ls], in1=state[lt, "zinvb"][:]
            )
        return go

    def piece_outproj(lt, g, gn):
        # one group: gn (<=3) jt-row-blocks: matmuls into one psc buf, one
        # grouped bf16 cast, one ring store
        def go():
            ls = bass.ts(lt, LT)
            ps = psc.tile([128, QB, LT], F32, tag="psc", name="ps_out")
            for i in range(gn):
                nc.tensor.matmul(
                    ps[:, i, :],
                    lhsT=wos[:, bass.ts(3 * g + i, JT)],
                    rhs=attnTb[:, ls],
                    start=True,
                    stop=True,
                )
            st = stage.tile([128, QB, LT], BF16, tag="st", name="st")
            nc.vector.tensor_copy(out=st[:, 0:gn, :], in_=ps[:, 0:gn, :])
            dst = outT[3 * g * JT : (3 * g + gn) * JT, ls].rearrange(
                "(n p) l -> p n l", p=128
            )
            nc.gpsimd.dma_start(dst, st[:, 0:gn, :])
        return go

    def drip():
        if pending:
            pending[0][0] -= 1
            if pending[0][0] < 0:
                pending.pop(0)[1]()

    qk_proj(QTb, wqs, qTs, 0)
    qk_proj(KTb, wks, kTs, 0)

    # carried attn-emission queue (lag-2 software pipeline on the PE)
    attnq = []

    for lt in range(NLT):
        ls = bass.ts(lt, LT)
        zseg = zpool.tile([128, HPC * LT], F32, tag="zseg", name=f"zseg_{lt}")
        for h in range(HPC):
            hd = bass.ts(h, HEAD_DIM)
            nxt = lt * HPC + h + 2  # prefetch two head-blocks ahead
            if nxt < NLT * HPC:
                mask_fetch(nxt // HPC, nxt % HPC)
            maskc = state[lt, h, "maskc"]
            pa = psa.tile([128, LT], F32, tag="psa", name=f"psa_{lt}_{h}")
            mt0 = 0
            for qi, qn in enumerate(QUADS):
                # scores + fp8 mask-add for this quad
                ss = psc.tile([128, QB, LT], F32, tag="psc", name="ss")
                for i in range(qn):
                    nc.tensor.matmul(
                        ss[:, i, :],
                        lhsT=KTb[hd, bass.ts(mt0 + i, MT)],
                        rhs=QTb[hd, ls],
                        start=True,
                        stop=False,
                    )
                for i in range(qn):
                    nc.tensor.matmul(
                        ss[:, i, :],
                        lhsT=negI[:],
                        rhs=maskc[:, mt0 + i, :],
                        start=False,
                        stop=True,
                    )
                pT = ppool.tile([128, QB, LT], BF16, tag="pT", name="pT")
                nc.scalar.activation(
                    pT[:, :qn, :], ss[:, :qn, :], mybir.ActivationFunctionType.Exp
                )

                def make_attn(mt0=mt0, qn=qn, pT=pT, pa=pa, h=h):
                    def go():
                        for i in range(qn):
                            mt = mt0 + i
                            nc.tensor.matmul(
                                pa[:VROW, :],
                                lhsT=vaug[:, h, mt, :],
                                rhs=pT[:, i, :],
                                start=(mt == 0),
                                stop=(mt == NMT - 1),
                            )
                    return go

                attnq.append(make_attn())

                # PE fillers for this quad (before draining lagged attn)
                if lt == 0 and h == 0:
                    if 1 <= qi <= 3:
                        qk_proj(KTb, wks, kTs, qi)
                    if qi >= 1:
                        v_proj_tri(3 * (qi - 1), 3)
                        if qi == 5:
                            v_proj_tri(15, 1)
                elif lt == 0 and h == 1:
                    if 1 <= qi <= 3:
                        qk_proj(QTb, wqs, qTs, qi)
                else:
                    drip()

                # lag-2 drain of the attn pipeline
                while len(attnq) > ATTN_LAG:
                    attnq.pop(0)()
                mt0 += qn
            # head epilogue: attnT + Z copies (DVE; runs behind the last attn)
            def head_copies(lt=lt, h=h, pa=pa, hd=hd, ls=ls, zseg=zseg):
                def go():
                    nc.vector.tensor_copy(out=attnTb[hd, ls], in_=pa[0:HEAD_DIM, :])
                    nc.vector.tensor_copy(
                        out=zseg[HEAD_DIM : HEAD_DIM + 1, bass.ts(h, LT)],
                        in_=pa[HEAD_DIM : HEAD_DIM + 1, :],
                    )
                return go
            attnq.append(head_copies())

        # drain through this l-tile's last copies before the z store
        while attnq:
            attnq.pop(0)()
        nc.sync.dma_start(zdram[lt][None, :], zseg[HEAD_DIM : HEAD_DIM + 1, :])
        sp = 3 if lt == 2 else 1
        pending.append([1, piece_zload(lt)])
        pending.append([0, piece_recip_bcast(lt)])
        pending.append([sp, piece_norm(lt)])
        pending.append([sp, piece_outproj(lt, 0, 3)])
        pending.append([sp, piece_outproj(lt, 1, 3)])
        pending.append([sp, piece_outproj(lt, 2, 2)])

    while pending:
        pending.pop(0)[1]()


_CACHE = {}


def _build():
    if "nc" in _CACHE:
        return _CACHE["nc"]
    nc = bacc.Bacc("TRN2", target_bir_lowering=False, debug=False,
                   num_devices=NCORES)
    qT = nc.dram_tensor("qT", [EMB, L], BF16, kind="ExternalInput").ap()
    kT = nc.dram_tensor("kT", [EMB, L], BF16, kind="ExternalInput").ap()
    vT = nc.dram_tensor("vT", [EMB, L], BF16, kind="ExternalInput").ap()
    wqT = nc.dram_tensor("wqT", [128, NET, ROWS], BF16, kind="ExternalInput").ap()
    wkT = nc.dram_tensor("wkT", [128, NET, ROWS], BF16, kind="ExternalInput").ap()
    wvT = nc.dram_tensor("wvT", [128, NET, ROWS], BF16, kind="ExternalInput").ap()
    woT = nc.dram_tensor("woT", [ROWS, EMB], BF16, kind="ExternalInput").ap()
    maskT = nc.dram_tensor("maskT", [HPC, L, L], FP8, kind="ExternalInput").ap()
    negIT = nc.dram_tensor("negIT", [128, 128], FP8, kind="ExternalInput").ap()
    outT = nc.dram_tensor("outT", [EMB, L], BF16, kind="ExternalOutput").ap()

    with tile.TileContext(nc) as tc:
        _mha_kernel(tc, outT, qT, kT, vT, wqT, wkT, wvT, woT, maskT, negIT)
    nc.compile()
    _CACHE["nc"] = nc
    return nc


def _pack_w(w):
    # [ROWS, EMB] -> w.T [EMB, ROWS] -> [128, NET, ROWS] with e = o*128+p
    return np.ascontiguousarray(
        w.T.reshape(NET, 128, ROWS).transpose(1, 0, 2)
    ).astype(NPBF16)


_NEGI = (MASK_NEG * np.eye(128, dtype=np.float32)).astype(NPFP8)


def _prep_in_maps(q, k, v, mask, Wq, Wk, Wv, Wo):
    qT = np.ascontiguousarray(q.T).astype(NPBF16)
    kT = np.ascontiguousarray(k.T).astype(NPBF16)
    vT = np.ascontiguousarray(v.T).astype(NPBF16)
    in_maps = []
    for c in range(NCORES):
        rows = slice(c * ROWS, (c + 1) * ROWS)
        # fp8 {0,1} mask, 1 = masked: bytes 0x00 / 0x38 (= fp8e4m3 1.0)
        mT = np.ascontiguousarray(
            mask[c * HPC : (c + 1) * HPC].swapaxes(1, 2)
        ).view(np.uint8) * np.uint8(0x38)
        in_maps.append({
            "qT": qT,
            "kT": kT,
            "vT": vT,
            "wqT": _pack_w(Wq[rows] * SCALE),
            "wkT": _pack_w(Wk[rows]),
            "wvT": _pack_w(Wv[rows]),
            "woT": np.ascontiguousarray(Wo[:, rows].T).astype(NPBF16),
            "maskT": mT.view(NPFP8),
            "negIT": _NEGI,
        })
    return in_maps


def run(q, k, v, mask, Wq, Wk, Wv, Wo, **spmd_kwargs):
    nc = _build()
    in_maps = _prep_in_maps(q, k, v, mask, Wq, Wk, Wv, Wo)
    res = run_bass_kernel_spmd(nc, in_maps, list(range(NCORES)), **spmd_kwargs)
    outT = np.zeros((EMB, L), np.float32)
    for r in res.results:
        outT += np.asarray(r["outT"], dtype=np.float32)
    out = np.ascontiguousarray(outT.T)
    return out, res


def kernel(q, k, v, mask, Wq, Wk, Wv, Wo):
    q, k, v = (np.asarray(x, np.float32) for x in (q, k, v))
    Wq, Wk, Wv, Wo = (np.asarray(x, np.float32) for x in (Wq, Wk, Wv, Wo))
    mask = np.asarray(mask, bool)
    out, _ = run(q, k, v, mask, Wq, Wk, Wv, Wo)
    return out
